# revision 3
# baseline (speedup 1.0000x reference)
"""Grouped-Query Attention (B=2, S=2048, E=2048, H=16, KVH=4, D=128, causal)
as a Bass/Tile kernel on 8 Trainium2 NeuronCores.

Sharding: core c handles batch b=c//4 and kv-head-group g=c%4 (4 q heads +
1 kv head per core).  Out-proj is row-sharded: each core computes a partial
[E,S] (transposed) output; host sums the 4 partials per batch.

All activations/weights are kept TRANSPOSED (feature-major) so every matmul
has its contraction dim on SBUF partitions with no on-chip transposes
(except V, which needs [S,D] layout for the PV matmul - done via 16 cheap
PE transposes).  Scores are computed in [k,q] orientation so exp output
feeds the PV matmul directly; the softmax denominator comes from a
ones-vector matmul; normalization is applied after PV (deferred division).

Matmul inputs are bf16 (fp32 PSUM accumulation); measured end-to-end
absmax-relative error vs the fp32 reference is ~3.4e-3.
"""
import sys

for _p in ("/opt/trn_rl_repo", "/root/.axon_site/_ro/trn_rl_repo"):
    if _p not in sys.path:
        sys.path.append(_p)

import numpy as np
import ml_dtypes

import concourse.bass as bass
import concourse.mybir as mybir
import concourse.tile as tile
from concourse import bacc, bass_utils

B, S, E = 2, 2048, 2048
H, KVH = 16, 4
D = E // H              # 128
G = H // KVH            # 4 q heads per kv head
HPC = H // 8 * 2        # 4 q heads per core
SCALE = 1.0 / float(np.sqrt(D))
P = 128                 # partitions
NQ = 512                # q-group width (moving N)
BF = mybir.dt.bfloat16
F32 = mybir.dt.float32

_CACHE = {}


def _build():
    nc = bacc.Bacc("TRN2", target_bir_lowering=False, debug=False, num_devices=8)
    xT = nc.dram_tensor("xT", [E, S], BF, kind="ExternalInput").ap()
    wqkvT = nc.dram_tensor("wqkvT", [E, 768], BF, kind="ExternalInput").ap()
    woT = nc.dram_tensor("woT", [512, E], BF, kind="ExternalInput").ap()
    ident = nc.dram_tensor("ident", [P, P], BF, kind="ExternalInput").ap()
    ones = nc.dram_tensor("ones", [P, 1], BF, kind="ExternalInput").ap()
    tri = nc.dram_tensor("tri", [P, P], BF, kind="ExternalInput").ap()
    outT = nc.dram_tensor("outT", [E, S], F32, kind="ExternalOutput").ap()

    EK = E // P          # 16 contraction chunks for projections
    with tile.TileContext(nc) as tc:
        with tc.tile_pool(name="persist", bufs=1) as pp, \
             tc.tile_pool(name="probs", bufs=8) as prb, \
             tc.tile_pool(name="bcast", bufs=2) as bcp, \
             tc.tile_pool(name="small", bufs=2) as smp, \
             tc.tile_pool(name="outp", bufs=3) as outp, \
             tc.tile_pool(name="ps_proj", bufs=2, space="PSUM") as ps_proj, \
             tc.tile_pool(name="ps_s", bufs=2, space="PSUM") as ps_sp, \
             tc.tile_pool(name="ps_o", bufs=2, space="PSUM") as ps_op, \
             tc.tile_pool(name="ps_sum", bufs=2, space="PSUM") as ps_sump:

            # ---- load inputs ----
            xT_sb = []
            for i in range(EK):
                t = pp.tile([P, S], BF, tag=f"xT{i}")
                nc.sync.dma_start(out=t, in_=xT[i * P:(i + 1) * P, :])
                xT_sb.append(t)
            wqkv_sb = []
            for i in range(EK):
                t = pp.tile([P, 768], BF, tag=f"wq{i}")
                nc.sync.dma_start(out=t, in_=wqkvT[i * P:(i + 1) * P, :])
                wqkv_sb.append(t)
            wo_sb = []
            for j in range(4):
                t = pp.tile([P, E], BF, tag=f"wo{j}")
                nc.sync.dma_start(out=t, in_=woT[j * P:(j + 1) * P, :])
                wo_sb.append(t)
            id_sb = pp.tile([P, P], BF, tag="ident")
            nc.sync.dma_start(out=id_sb, in_=ident)
            ones_sb = pp.tile([P, 1], BF, tag="ones")
            nc.sync.dma_start(out=ones_sb, in_=ones)
            tri_sb = pp.tile([P, P], BF, tag="tri")
            nc.sync.dma_start(out=tri_sb, in_=tri)

            # ---- phase 1: qkvT[768, S] = WqkvT.T @ xT ----
            qkv_sb = [pp.tile([P, S], BF, tag=f"qkv{m}", name=f"qkv{m}") for m in range(6)]
            for m in range(6):
                for ng in range(S // NQ):
                    ps = ps_proj.tile([P, NQ], F32, tag="proj")
                    for ke in range(EK):
                        nc.tensor.matmul(
                            ps,
                            wqkv_sb[ke][:, m * P:(m + 1) * P],
                            xT_sb[ke][:, ng * NQ:(ng + 1) * NQ],
                            start=(ke == 0), stop=(ke == EK - 1))
                    # alternate copy engine to split the work
                    if ng % 2 == 0:
                        nc.scalar.copy(qkv_sb[m][:, ng * NQ:(ng + 1) * NQ], ps)
                    else:
                        nc.vector.tensor_copy(qkv_sb[m][:, ng * NQ:(ng + 1) * NQ], ps)
            kT = qkv_sb[4]          # [D, S]
            vT = qkv_sb[5]          # [D, S]

            # ---- phase 1b: v natural layout [S, D] via PE transposes ----
            v_sb = []
            for kc in range(S // P):
                pst = ps_sp.tile([P, P], BF, tag="s")
                nc.tensor.transpose(pst, vT[:, kc * P:(kc + 1) * P], id_sb)
                vt = pp.tile([P, D], BF, tag=f"v{kc}")
                nc.vector.tensor_copy(vt, pst)
                v_sb.append(vt)

            # ---- phase 2: attention (4 heads, q-groups of 512, causal) ----
            attn_sb = [pp.tile([P, S], BF, tag=f"at{h}", name=f"at{h}") for h in range(HPC)]
            for h in range(HPC):
                qT_h = qkv_sb[h]
                for g4 in range(S // NQ):
                    kmax = 4 * g4 + 4
                    ps_o = ps_op.tile([P, NQ], F32, tag="o")
                    ps_sum = ps_sump.tile([1, NQ], F32, tag="sum")
                    for kc in range(kmax):
                        ps_s = ps_sp.tile([P, NQ], F32, tag="s")
                        nc.tensor.matmul(
                            ps_s, kT[:, kc * P:(kc + 1) * P],
                            qT_h[:, g4 * NQ:(g4 + 1) * NQ],
                            start=True, stop=True)
                        pr = prb.tile([P, NQ], BF, tag="pr")
                        rel = P * (kc - 4 * g4)
                        if rel <= 0:
                            nc.scalar.activation(
                                pr, ps_s, mybir.ActivationFunctionType.Exp,
                                scale=SCALE)
                        else:
                            nc.gpsimd.memset(pr[:, :rel], 0.0)
                            nc.scalar.activation(
                                pr[:, rel:], ps_s[:, rel:],
                                mybir.ActivationFunctionType.Exp, scale=SCALE)
                        if rel >= 0:
                            # diagonal block: keep kp <= qp
                            nc.vector.tensor_mul(
                                pr[:, rel:rel + P], pr[:, rel:rel + P], tri_sb)
                        nc.tensor.matmul(ps_sum, ones_sb, pr,
                                         start=(kc == 0), stop=(kc == kmax - 1))
                        nc.tensor.matmul(ps_o, v_sb[kc], pr,
                                         start=(kc == 0), stop=(kc == kmax - 1))
                    rec = smp.tile([1, NQ], F32, tag="rec")
                    nc.vector.reciprocal(rec, ps_sum)
                    bc = bcp.tile([P, NQ], F32, tag="bc")
                    nc.gpsimd.partition_broadcast(bc, rec)
                    nc.vector.tensor_mul(
                        attn_sb[h][:, g4 * NQ:(g4 + 1) * NQ], ps_o, bc)

            # ---- phase 3: partial out-proj, outT[E, S] = WoT.T @ attnT ----
            for me in range(E // P):
                for ng in range(S // NQ):
                    ps = ps_proj.tile([P, NQ], F32, tag="proj")
                    for j in range(4):
                        nc.tensor.matmul(
                            ps, wo_sb[j][:, me * P:(me + 1) * P],
                            attn_sb[j][:, ng * NQ:(ng + 1) * NQ],
                            start=(j == 0), stop=(j == 3))
                    ot = outp.tile([P, NQ], F32, tag="out")
                    if ng % 2 == 0:
                        nc.scalar.copy(ot, ps)
                    else:
                        nc.vector.tensor_copy(ot, ps)
                    nc.sync.dma_start(
                        out=outT[me * P:(me + 1) * P, ng * NQ:(ng + 1) * NQ],
                        in_=ot)
    nc.compile()
    return nc


def _get_nc():
    if "nc" not in _CACHE:
        _CACHE["nc"] = _build()
    return _CACHE["nc"]


def kernel(x, Wq, Wk, Wv, Wo, _trace=False, _tmpdir=None):
    x = np.asarray(x, np.float32)
    Wq, Wk, Wv, Wo = (np.asarray(a, np.float32) for a in (Wq, Wk, Wv, Wo))
    nc = _get_nc()
    ident = np.eye(P, dtype=ml_dtypes.bfloat16)
    ones = np.ones((P, 1), dtype=ml_dtypes.bfloat16)
    tri = np.triu(np.ones((P, P), np.float32)).astype(ml_dtypes.bfloat16)
    in_maps = []
    for c in range(8):
        b, g = c // 4, c % 4
        wqkv = np.concatenate(
            [Wq[512 * g:512 * (g + 1)],
             Wk[128 * g:128 * (g + 1)],
             Wv[128 * g:128 * (g + 1)]], axis=0)
        in_maps.append({
            "xT": np.ascontiguousarray(x[b].T).astype(ml_dtypes.bfloat16),
            "wqkvT": np.ascontiguousarray(wqkv.T).astype(ml_dtypes.bfloat16),
            "woT": np.ascontiguousarray(Wo[:, 512 * g:512 * (g + 1)].T).astype(
                ml_dtypes.bfloat16),
            "ident": ident, "ones": ones, "tri": tri,
        })
    res = bass_utils.run_bass_kernel_spmd(
        nc, in_maps, core_ids=list(range(8)), trace=_trace, tmpdir=_tmpdir)
    out = np.zeros((B, S, E), np.float32)
    for c in range(8):
        out[c // 4] += res.results[c]["outT"].T
    if _trace:
        return out, res
    return out
